# revision 4
# baseline (speedup 1.0000x reference)
"""Trainium2 Bass kernel for nn_CrossAttnActGPT2Attention.

Math: the module is cross-attention from S=4096 query tokens to a KV
sequence of length 2 (a learned no-op token and one token projected from
`activation`).  Softmax over 2 keys is a sigmoid of the score difference,
so the whole module folds, per batch element b, into

    out[s, :] = c + sigmoid(x[s, :] @ G_b + e_b) @ U_b

with
    G_b[:, h] = W_q[:, h*64:(h+1)*64] @ (k1_b[h] - k0[h])      [D, H]
    e_b[h]    = b_q[h*64:(h+1)*64] . (k1_b[h] - k0[h])         [H]
    U_b[h, :] = (v1_b[h] - v0[h]) @ W_proj[h*64:(h+1)*64, :]   [H, D]
    c         = v0.flatten() @ W_proj + b_proj                 [D]
    (k1_b, v1_b from kv = activation[b] @ W_kv + b_kv; k0, v0 = no-op token)

This is exact (validated to ~7e-7 rel. Frobenius error vs the f32 jax
reference).  The per-batch G/U/e/c precompute is ~34 MFLOP and runs on
host; the device kernel streams the 128 MiB of x in and 128 MiB of out --
the memory-bound part -- sharded data-parallel, one batch element per
NeuronCore.

Device kernel per core (xT is x pre-transposed to [D, S] on host):
  per s-block of 512:
    dT[h, s]  = sum_c G[c-chunk, h]^T @ xT[c-chunk, s-block]   (PE, 8 accum MMs)
    sig[h, s] = sigmoid(dT + e)  (ACT), row 16 memset to 1.0
    per 128-col subtile t:
      out[s, :] = sig[:, t]^T(17x128) @ U(17x1024)             (PE, 2 MMs)
      copy PSUM -> SBUF (ACT/DVE), DMA out
"""

import numpy as np

import concourse.bass as bass
import concourse.tile as tile
from concourse import mybir
from concourse.bass_utils import run_bass_kernel_spmd
from concourse.vector_clock import ScopedClock

B, S, D, H, HD = 8, 4096, 1024, 16, 64
SBLK = 512           # s-columns per mm1 block (= max fp32 moving free dim)
NBLK = S // SBLK     # 8
NSUB = SBLK // 128   # 4 output subtiles per block
NCHUNK = D // 128    # 8 contraction chunks
F32 = mybir.dt.float32
MM_DT = mybir.dt.float32   # matmul operand dtype (float32 or float32r)


class _TileContextSplitDrain(tile.TileContext):
    """The walrus build here rejects >1 sync wait on a CTRL (drain)
    instruction; split the final drain's waits across single-wait NOPs."""

    def _drain_and_barrier(self, tick_clock, wait_clock):
        nc = self.nc
        probe = nc.sync.nop(nofuse=True, hint="drain_wait_probe")
        wait_clock.add_sem_waits(
            probe.ins, ScopedClock({None: tick_clock.global_clock})
        )
        si = probe.ins.sync_info
        waits = list(si.on_wait or []) if si is not None else []
        if len(waits) > 1:
            si.on_wait = [waits[0]]
            for w in waits[1:]:
                extra = nc.sync.nop(nofuse=True, hint="drain_wait_split")
                extra.ins.sync_info = type(si)(on_wait=[w], on_update=[])
        nc.sync.drain()
        nc.all_engine_barrier()
        assert self.sems is not None
        popped = nc._tile_sem_poison_stack.pop()
        assert popped is self._sem_poison
        nc.clear_and_free_semaphores(list(self.sems.allocated().values()))
        nc.all_engine_barrier()


def _split_multi_waits(nc):
    """Walrus here allows at most one sync-wait per instruction.  Move
    extra waits of any instruction onto same-engine NOPs placed directly
    before it (same sequencer => identical blocking semantics)."""
    n_split = 0
    for bb in nc.main_func.blocks:
        insts = list(bb.instructions)
        new_list = []
        changed = False
        for inst in insts:
            si = inst.sync_info
            waits = list(si.on_wait) if (si is not None and si.on_wait) else []
            if len(waits) > 1:
                changed = True
                for k, w in enumerate(waits[:-1]):
                    nop = mybir.InstNoOp(
                        name=f"{inst.name}-ws{k}", ins=[], outs=[]
                    )
                    nop.engine = inst.engine
                    nop.sync_info = type(si)(on_wait=[w], on_update=[])
                    nc.register_instruction(nop)
                    new_list.append(nop)
                    n_split += 1
                si.on_wait = [waits[-1]]
            new_list.append(inst)
        if changed:
            bb.instructions = new_list
    return n_split


def _build_kernel():
    nc = bass.Bass("TRN2", target_bir_lowering=False, debug=False, num_devices=B)

    xT = nc.dram_tensor("xT", [D, S], F32, kind="ExternalInput")
    G = nc.dram_tensor("G", [D, H], F32, kind="ExternalInput")
    U = nc.dram_tensor("U", [H + 1, D], F32, kind="ExternalInput")
    e = nc.dram_tensor("e", [H, 1], F32, kind="ExternalInput")
    out = nc.dram_tensor("out", [S, D], F32, kind="ExternalOutput")

    # [D, S] -> [p, chunk, s];  [S, D] -> [blk, p, t, j]
    xT_v = xT.ap().rearrange("(c p) s -> p c s", p=128)
    out_v = out.ap().rearrange("(blk t p) j -> blk p t j", t=NSUB, p=128)

    with _TileContextSplitDrain(nc) as tc:
        with (
            tc.tile_pool(name="singles", bufs=1) as singles,
            tc.tile_pool(name="xt", bufs=3) as xt_pool,
            tc.tile_pool(name="sig", bufs=3) as sig_pool,
            tc.tile_pool(name="osb", bufs=3) as out_pool,
            tc.tile_pool(name="pd", bufs=2, space="PSUM") as pd_pool,
            tc.tile_pool(name="po", bufs=4, space="PSUM") as po_pool,
        ):
            g_sb = singles.tile([128, NCHUNK, H], F32)
            u_sb = singles.tile([H + 1, D], F32)
            e_sb = singles.tile([H, 1], F32)
            nc.sync.dma_start(out=g_sb, in_=G.ap().rearrange("(c p) h -> p c h", p=128))
            nc.sync.dma_start(out=u_sb, in_=U.ap())
            nc.sync.dma_start(out=e_sb, in_=e.ap())

            for blk in range(NBLK):
                xt_sb = xt_pool.tile([128, NCHUNK, SBLK], F32)
                nc.sync.dma_start(out=xt_sb, in_=xT_v[:, :, blk * SBLK:(blk + 1) * SBLK])

                pd = pd_pool.tile([H, SBLK], F32)
                for c in range(NCHUNK):
                    nc.tensor.matmul(
                        pd,
                        g_sb[:, c, :].bitcast(MM_DT),
                        xt_sb[:, c, :].bitcast(MM_DT),
                        start=(c == 0),
                        stop=(c == NCHUNK - 1),
                    )

                sig = sig_pool.tile([H + 1, SBLK], F32)
                nc.vector.memset(sig, 1.0)
                nc.scalar.activation(
                    out=sig[0:H, :],
                    in_=pd,
                    func=mybir.ActivationFunctionType.Sigmoid,
                    bias=e_sb,
                    scale=1.0,
                )

                osb = out_pool.tile([128, NSUB, D], F32)
                for t in range(NSUB):
                    lhsT = sig[:, t * 128:(t + 1) * 128].bitcast(MM_DT)
                    for half in range(2):
                        po = po_pool.tile([128, 512], F32)
                        nc.tensor.matmul(
                            po,
                            lhsT,
                            u_sb[:, half * 512:(half + 1) * 512].bitcast(MM_DT),
                            start=True,
                            stop=True,
                        )
                        dst = osb[:, t, half * 512:(half + 1) * 512]
                        if half == 0:
                            nc.scalar.copy(dst, po)
                        else:
                            nc.vector.tensor_copy(dst, po)
                nc.sync.dma_start(out=out_v[blk], in_=osb)

    _split_multi_waits(nc)
    return nc


_NC_CACHE = None


def _get_nc():
    global _NC_CACHE
    if _NC_CACHE is None:
        _NC_CACHE = _build_kernel()
    return _NC_CACHE


def _host_precompute(activation, W_q, b_q, W_kv, b_kv, no_op_k, no_op_v,
                     W_proj, b_proj):
    """Per-batch G [B,D,H], U' [B,H+1,D] (last row = c), e [B,H,1] in f64."""
    act = activation.astype(np.float64)
    W_q = W_q.astype(np.float64)
    b_q = b_q.astype(np.float64)
    W_kv = W_kv.astype(np.float64)
    b_kv = b_kv.astype(np.float64)
    k0 = no_op_k.astype(np.float64).reshape(H, HD)
    v0 = no_op_v.astype(np.float64).reshape(H, HD)
    W_p = W_proj.astype(np.float64)
    b_p = b_proj.astype(np.float64)

    kv = act @ W_kv + b_kv
    k1 = kv[:, :D].reshape(B, H, HD)
    v1 = kv[:, D:].reshape(B, H, HD)
    dk = k1 - k0[None]
    dv = v1 - v0[None]
    G = np.einsum("dhe,bhe->bdh", W_q.reshape(D, H, HD), dk)
    e = np.einsum("he,bhe->bh", b_q.reshape(H, HD), dk)
    U = np.einsum("bhe,hej->bhj", dv, W_p.reshape(H, HD, D))
    c = v0.reshape(-1) @ W_p + b_p
    Up = np.concatenate([U, np.broadcast_to(c, (B, 1, D))], axis=1)
    return (G.astype(np.float32), Up.astype(np.float32),
            e.astype(np.float32)[:, :, None])


def kernel(hidden_states, activation, W_q, b_q, W_kv, b_kv, no_op_k, no_op_v,
           W_proj, b_proj):
    G, Up, e = _host_precompute(activation, W_q, b_q, W_kv, b_kv,
                                no_op_k, no_op_v, W_proj, b_proj)
    xT = np.ascontiguousarray(
        np.asarray(hidden_states, dtype=np.float32).transpose(0, 2, 1)
    )
    nc = _get_nc()
    in_maps = [
        {"xT": xT[b], "G": np.ascontiguousarray(G[b]),
         "U": np.ascontiguousarray(Up[b]), "e": np.ascontiguousarray(e[b])}
        for b in range(B)
    ]
    res = run_bass_kernel_spmd(nc, in_maps, core_ids=list(range(B)))
    return np.stack([res.results[b]["out"] for b in range(B)], axis=0)


# revision 13
# speedup vs baseline: 136.1187x; 136.1187x over previous
"""Trainium2 Bass kernel for nn_CrossAttnActGPT2Attention.

Math: the module is cross-attention from S=4096 query tokens to a KV
sequence of length 2 (a learned no-op token and one token projected from
`activation`).  Softmax over 2 keys is a sigmoid of the score difference,
so the whole module folds, per batch element b, into

    out[s, :] = c + sigmoid(x[s, :] @ G_b + e_b) @ U_b

with
    G_b[:, h] = W_q[:, h*64:(h+1)*64] @ (k1_b[h] - k0[h])      [D, H]
    e_b[h]    = b_q[h*64:(h+1)*64] . (k1_b[h] - k0[h])         [H]
    U_b[h, :] = (v1_b[h] - v0[h]) @ W_proj[h*64:(h+1)*64, :]   [H, D]
    c         = v0.flatten() @ W_proj + b_proj                 [D]
    (k1_b, v1_b from kv = activation[b] @ W_kv + b_kv; k0, v0 = no-op token)

This is exact (validated to ~7e-7 rel. Frobenius error vs the f32 jax
reference).  The per-batch G/U/e/c precompute is ~34 MFLOP and runs on
host; the device kernel streams the 128 MiB of x in and 128 MiB of out --
the memory-bound part -- sharded data-parallel, one batch element per
NeuronCore.

Device kernel per core (xT is x pre-transposed to [D, S] on host).
G/U/e are host-packed with 4 replicas at partition offsets 0/32/64/96
(G as columns of the stationary operand, so mm1 directly produces the
sigmoid input replicated 4x; a 17th zero G column with bias 30 makes
sigmoid row 16 == 1.0, the homogeneous row that adds the constant c
via U's last row).  Per s-block of 512:
    pd[128, s] = sum_c Gq[c-chunk, 0:113]^T @ xT[c-chunk, s-block]
                                                  (PE, 8 accum MMs, M=113)
    sig[128, s] = sigmoid(pd + e)                 (ACT, one op)
    per 128-col subtile t, half n:
      out[t-rows, n-cols] = sig[32j:32j+17, t]^T @ Uq[32j:32j+17, n]
        -- tile_position=(32j, 0), j cycling 0..3: the eight K=17
           matmuls of a block run 4-way concurrent on row groups
      copy PSUM -> SBUF (ACT/DVE), DMA out
"""

import numpy as np

import concourse.bass as bass
import concourse.tile as tile
from concourse import mybir
from concourse.bass_utils import run_bass_kernel_spmd
from concourse.vector_clock import ScopedClock

B, S, D, H, HD = 8, 4096, 1024, 16, 64
SBLK = 512           # s-columns per mm1 block (= max fp32 moving free dim)
NBLK = S // SBLK     # 8
NSUB = SBLK // 128   # 4 output subtiles per block
NCHUNK = D // 128    # 8 contraction chunks
F32 = mybir.dt.float32
MM_DT = mybir.dt.float32   # matmul operand dtype (float32 or float32r)


class _TileContextSplitDrain(tile.TileContext):
    """The walrus build here rejects >1 sync wait on a CTRL (drain)
    instruction; split the final drain's waits across single-wait NOPs."""

    def _drain_and_barrier(self, tick_clock, wait_clock):
        nc = self.nc
        probe = nc.sync.nop(nofuse=True, hint="drain_wait_probe")
        wait_clock.add_sem_waits(
            probe.ins, ScopedClock({None: tick_clock.global_clock})
        )
        si = probe.ins.sync_info
        waits = list(si.on_wait or []) if si is not None else []
        if len(waits) > 1:
            si.on_wait = [waits[0]]
            for w in waits[1:]:
                extra = nc.sync.nop(nofuse=True, hint="drain_wait_split")
                extra.ins.sync_info = type(si)(on_wait=[w], on_update=[])
        nc.sync.drain()
        nc.all_engine_barrier()
        assert self.sems is not None
        popped = nc._tile_sem_poison_stack.pop()
        assert popped is self._sem_poison
        nc.clear_and_free_semaphores(list(self.sems.allocated().values()))
        nc.all_engine_barrier()


def _split_multi_waits(nc):
    """Walrus here allows at most one sync-wait per instruction.  Move
    extra waits of any instruction onto same-engine NOPs placed directly
    before it (same sequencer => identical blocking semantics)."""
    n_split = 0
    for bb in nc.main_func.blocks:
        insts = list(bb.instructions)
        new_list = []
        changed = False
        for inst in insts:
            si = inst.sync_info
            waits = list(si.on_wait) if (si is not None and si.on_wait) else []
            if len(waits) > 1:
                changed = True
                for k, w in enumerate(waits[:-1]):
                    nop = mybir.InstNoOp(
                        name=f"{inst.name}-ws{k}", ins=[], outs=[]
                    )
                    nop.engine = inst.engine
                    nop.sync_info = type(si)(on_wait=[w], on_update=[])
                    nc.register_instruction(nop)
                    new_list.append(nop)
                    n_split += 1
                si.on_wait = [waits[-1]]
            new_list.append(inst)
        if changed:
            bb.instructions = new_list
    return n_split


def _build_kernel():
    nc = bass.Bass("TRN2", target_bir_lowering=False, debug=False, num_devices=B)

    xT = nc.dram_tensor("xT", [D, S], MM_DT, kind="ExternalInput")
    G = nc.dram_tensor("G", [D, 128], MM_DT, kind="ExternalInput")
    U = nc.dram_tensor("U", [128, D], MM_DT, kind="ExternalInput")
    e = nc.dram_tensor("e", [128, 1], F32, kind="ExternalInput")
    out = nc.dram_tensor("out", [S, D], F32, kind="ExternalOutput")

    # [D, S] -> [p, chunk, s];  [S, D] -> [blk, p, t, j]
    xT_v = xT.ap().rearrange("(c p) s -> p c s", p=128)
    out_v = out.ap().rearrange("(blk t p) j -> blk p t j", t=NSUB, p=128)

    with _TileContextSplitDrain(nc) as tc:
        with (
            tc.tile_pool(name="singles", bufs=1) as singles,
            tc.tile_pool(name="xt", bufs=4) as xt_pool,
            tc.tile_pool(name="sig", bufs=3) as sig_pool,
            tc.tile_pool(name="osb", bufs=3) as out_pool,
            tc.tile_pool(name="pd", bufs=2, space="PSUM") as pd_pool,
            tc.tile_pool(name="po", bufs=6, space="PSUM") as po_pool,
        ):
            g_sb = singles.tile([128, NCHUNK, 128], MM_DT)
            u_sb = singles.tile([128, D], MM_DT)
            e_sb = singles.tile([128, 1], F32)
            nc.sync.dma_start(out=g_sb, in_=G.ap().rearrange("(c p) h -> p c h", p=128))
            nc.sync.dma_start(out=u_sb, in_=U.ap())
            nc.sync.dma_start(out=e_sb, in_=e.ap())

            for blk in range(NBLK):
                xt_sb = xt_pool.tile([128, NCHUNK, SBLK], MM_DT)
                nc.sync.dma_start(out=xt_sb, in_=xT_v[:, :, blk * SBLK:(blk + 1) * SBLK])

                pd = pd_pool.tile([128, SBLK], F32)
                for c in range(NCHUNK):
                    nc.tensor.matmul(
                        pd[0:113, :],
                        g_sb[:, c, 0:113],
                        xt_sb[:, c, :],
                        start=(c == 0),
                        stop=(c == NCHUNK - 1),
                    )

                sig = sig_pool.tile([128, SBLK], MM_DT)
                nc.scalar.activation(
                    out=sig[0:113, :],
                    in_=pd[0:113, :],
                    func=mybir.ActivationFunctionType.Sigmoid,
                    bias=e_sb[0:113, :],
                    scale=1.0,
                )

                osb = out_pool.tile([128, NSUB, D], F32)
                mm2 = [(t, half) for t in range(NSUB) for half in range(2)]
                for idx, (t, half) in enumerate(mm2):
                    j = idx % 4
                    po = po_pool.tile([128, 512], F32)
                    nc.tensor.matmul(
                        po,
                        sig[32 * j:32 * j + H + 1, t * 128:(t + 1) * 128],
                        u_sb[32 * j:32 * j + H + 1,
                             half * 512:(half + 1) * 512],
                        start=True,
                        stop=True,
                        tile_position=(32 * j, 0),
                    )
                    dst = osb[:, t, half * 512:(half + 1) * 512]
                    if half == 0:
                        nc.scalar.copy(dst, po)
                    else:
                        nc.vector.tensor_copy(dst, po)
                nc.sync.dma_start(out=out_v[blk], in_=osb)

    _split_multi_waits(nc)
    return nc


_NC_CACHE = None


def _get_nc():
    global _NC_CACHE
    if _NC_CACHE is None:
        _NC_CACHE = _build_kernel()
    return _NC_CACHE


def _host_precompute(activation, W_q, b_q, W_kv, b_kv, no_op_k, no_op_v,
                     W_proj, b_proj):
    """Per-batch G [B,D,H], U' [B,H+1,D] (last row = c), e [B,H,1] in f64."""
    act = activation.astype(np.float64)
    W_q = W_q.astype(np.float64)
    b_q = b_q.astype(np.float64)
    W_kv = W_kv.astype(np.float64)
    b_kv = b_kv.astype(np.float64)
    k0 = no_op_k.astype(np.float64).reshape(H, HD)
    v0 = no_op_v.astype(np.float64).reshape(H, HD)
    W_p = W_proj.astype(np.float64)
    b_p = b_proj.astype(np.float64)

    kv = act @ W_kv + b_kv
    k1 = kv[:, :D].reshape(B, H, HD)
    v1 = kv[:, D:].reshape(B, H, HD)
    dk = k1 - k0[None]
    dv = v1 - v0[None]
    G = np.einsum("dhe,bhe->bdh", W_q.reshape(D, H, HD), dk)
    e = np.einsum("he,bhe->bh", b_q.reshape(H, HD), dk)
    U = np.einsum("bhe,hej->bhj", dv, W_p.reshape(H, HD, D))
    c = v0.reshape(-1) @ W_p + b_p
    Up = np.concatenate([U, np.broadcast_to(c, (B, 1, D))], axis=1)
    # 17th "homogeneous" channel: zero G column + bias 30 -> sigmoid == 1.0,
    # which multiplies U's last row (= c) in mm2.  Replicate all three at
    # partition offsets 0/32/64/96 so mm2 can row-tile 4-way.
    Gq = np.zeros((B, D, 128))
    Uq = np.zeros((B, 128, D))
    eq = np.zeros((B, 128))
    for j in range(4):
        Gq[:, :, 32 * j:32 * j + H] = G
        Uq[:, 32 * j:32 * j + H + 1] = Up
        eq[:, 32 * j:32 * j + H] = e
        eq[:, 32 * j + H] = 30.0
    return (Gq.astype(np.float32), Uq.astype(np.float32),
            eq.astype(np.float32)[:, :, None])


def kernel(hidden_states, activation, W_q, b_q, W_kv, b_kv, no_op_k, no_op_v,
           W_proj, b_proj):
    hidden_states = np.asarray(hidden_states)
    activation = np.asarray(activation)
    W_q, b_q = np.asarray(W_q), np.asarray(b_q)
    W_kv, b_kv = np.asarray(W_kv), np.asarray(b_kv)
    no_op_k, no_op_v = np.asarray(no_op_k), np.asarray(no_op_v)
    W_proj, b_proj = np.asarray(W_proj), np.asarray(b_proj)
    G, Up, e = _host_precompute(activation, W_q, b_q, W_kv, b_kv,
                                no_op_k, no_op_v, W_proj, b_proj)
    xT = np.ascontiguousarray(
        np.asarray(hidden_states, dtype=np.float32).transpose(0, 2, 1)
    )
    nc = _get_nc()
    in_maps = [
        {"xT": xT[b], "G": np.ascontiguousarray(G[b]),
         "U": np.ascontiguousarray(Up[b]), "e": np.ascontiguousarray(e[b])}
        for b in range(B)
    ]
    res = run_bass_kernel_spmd(nc, in_maps, core_ids=list(range(B)))
    return np.stack([res.results[b]["out"] for b in range(B)], axis=0)
